# revision 3
# baseline (speedup 1.0000x reference)
"""CAAM kernel for Trainium2: builder + host-side prep.

Per-core: one batch element. Layouts:
  x resident as 4 SBUF tiles [128, 8192]  (c-chunk, h*128+w)
  cam/E  [19, 8192]  (class on partitions; exp in place; slot reused as phase-C
  scratch, then as normalized attention weights Eall)
  stack  [(k*8+n), c] in one tile: A rows 0:128 cols 0:512, B rows 0:24 cols 512:1024
  keyT [128i, 2*152] + val [19, 256] packed in one tile
  attention per bin: qT [128, 2*1024], aff-exp normalized into Eall
  y per (bin, cc, nh): psum [128, 512]; BN stats allreduced; apply in F.
All small constants are host-packed into wpackE/wpackL column maps.
"""

import numpy as np
import concourse.bass as bass
import concourse.mybir as mybir

F32 = mybir.dt.float32
AX = mybir.AxisListType
OP = mybir.AluOpType
ACT = mybir.ActivationFunctionType

B, C, H, W = 8, 512, 64, 128
K, BH, BW = 19, 2, 4
NB = BH * BW          # 8
CI = C // 2           # 256
HWp = H * W           # 8192
RH, RW = H // BH, W // BW   # 32, 32
P = RH * RW           # 1024
CC = C // 128         # 4
IC = CI // 128        # 2
KN = K * NB           # 152
EPS = 1e-5

# -------- wpackE column map (early consts) --------
# GCN stack layout: partition p = n*19 + k  (chunk0: p<128, chunk1: p-128 < 24)
E_IDN = 0        # 128 cols               identity
E_WCAM = 128     # 76 cols (4 chunks x 19)
E_W1NK0 = 204    # 152 cols, rows 0:128   conv1 lhsT chunk0
E_W1NK1 = 356    # 152 cols, rows 0:24    conv1 lhsT chunk1
E_FNK0 = 508     # 19 cols, rows 0:128    fuse lhsT chunk0
E_FNK1 = 527     # 19 cols, rows 0:24     fuse lhsT chunk1
E_GANK = 546     # 2 cols: gcn_a-1 per stack row (chunk0, chunk1)
E_CAMB = 548     # 1 col, rows 0:19
E_FB = 549       # 1 col, rows 0:19       fuse_b
E_RAM1 = 550     # 1 col, rows 0:19       relu_a - 1
E_KB = 551       # 2 cols                 k_b chunks
E_ONE191 = 553   # 1 col, rows 0:19       ones
E_VB = 554       # 256 cols, row 0        v_b
E_ONE119 = 810   # 19 cols, row 0         ones
NE = 829

# -------- wpackL column map (late consts, [128, 35]) --------
L_QB = 0         # 2 cols
L_GAMMA = 2      # 4
L_BETA = 6       # 4
L_OAM1 = 10      # 4  out_a - 1
L_EPS = 14       # 1
L_ONE191 = 15    # 1, rows 0:19
L_ONE119 = 16    # 19, row 0
NL = 35

# -------- dsmallA ([128, 40]): phase A stats --------
A_CSUM = 0       # 8 cols, rows 0:19
A_ESUM = 8
A_CLS = 16
A_REC = 24
A_SCALE = 32     # 8 cols: cls * rec
NA = 40

# -------- dsmallDE ([128, 646]) --------
D_RS = 0         # 16: attnT row sums (ic, bin)
D_SQ = 16        # 64: y^2 sums (cc, bin, nh)
D_ST = 80        # 8: packed allreduce input (sum, sumsq per cc)
D_SBN = 88       # 8: allreduce output
D_SCOL = 96      # 4
D_BCOL = 100     # 4
D_RSUM = 104     # 2
D_MOM = 106      # 8
D_VAR = 114      # 4
D_MUSQ = 118     # 4
D_SD = 122       # 4
D_RSTD = 126     # 4
D_NSC = 130      # 4
D_RROW = 134     # 512, row 0
ND = 646

# -------- scrC column map (phase-C scratch inside bigE slot) --------
S_VA = 0         # 512   prelu'd t, chunk0
S_VB = 512       # 512, rows 0:24  chunk1
S_UG = 1024      # 512
S_MG = 1536      # 512
S_TT = 2048      # 608 = 4 x 152
S_L2A = 2656     # 512
S_L2B = 3168     # 512, rows 0:24
S_GL = 3680      # 512, rows 0:19  glob (prelu'd)
S_UG2 = 4192     # 512
S_MG2 = 4704     # 512
S_GT = 5216      # 76 = 4 x 19
S_L2T = 5292     # 608
NS = 5900

# attw pack: keyT 0:304, val 304:560
AT_KEYT = 0
AT_VAL = 304
NAT = 560


def host_prep(wts: dict) -> dict:
    w1 = np.asarray(wts["gcn_w1"], np.float32)
    ga = np.asarray(wts["gcn_a"], np.float32)
    fw = np.asarray(wts["fuse_w"], np.float32).reshape(-1)
    fb = float(np.asarray(wts["fuse_b"], np.float32).reshape(-1)[0])
    ra = float(np.asarray(wts["relu_a"], np.float32).reshape(-1)[0])

    wE = np.zeros((128, NE), np.float32)
    # conv1 lhsT: W[(m*19+kp), (n*19+k)] = w1[n, m] * (kp == k)
    W1NK = np.zeros((KN, KN), np.float32)
    FNK = np.zeros((KN, K), np.float32)
    ga_nk = np.zeros(KN, np.float32)
    for n in range(NB):
        for k in range(K):
            for m in range(NB):
                W1NK[m*K + k, n*K + k] = w1[n, m]
            FNK[n*K + k, k] = fw[n]
            ga_nk[n*K + k] = ga[n] - 1.0
    wE[:, E_W1NK0:E_W1NK0 + KN] = W1NK[0:128]
    wE[0:24, E_W1NK1:E_W1NK1 + KN] = W1NK[128:KN]
    wE[:, E_FNK0:E_FNK0 + K] = FNK[0:128]
    wE[0:24, E_FNK1:E_FNK1 + K] = FNK[128:KN]
    wE[:, E_GANK] = ga_nk[0:128]
    wE[0:24, E_GANK + 1] = ga_nk[128:KN]
    wE[:, E_IDN:E_IDN + 128] = np.eye(128, dtype=np.float32)
    wcamT = np.asarray(wts["conv_cam_w"], np.float32).T    # [512, 19]
    for cc in range(CC):
        wE[:, E_WCAM + cc*K:E_WCAM + (cc+1)*K] = wcamT[cc*128:(cc+1)*128]
    wE[0:K, E_CAMB] = np.asarray(wts["conv_cam_b"], np.float32)
    wE[0:K, E_FB] = fb
    wE[0:K, E_RAM1] = ra - 1.0
    wE[:, E_KB:E_KB + 2] = np.asarray(wts["k_b"], np.float32).reshape(IC, 128).T
    wE[0:K, E_ONE191] = 1.0
    wE[0, E_VB:E_VB + CI] = np.asarray(wts["v_b"], np.float32)
    wE[0, E_ONE119:E_ONE119 + K] = 1.0

    wL = np.zeros((128, NL), np.float32)
    wL[:, L_QB:L_QB + 2] = np.asarray(wts["q_b"], np.float32).reshape(IC, 128).T
    wL[:, L_GAMMA:L_GAMMA + 4] = np.asarray(wts["bn_gamma"], np.float32).reshape(CC, 128).T
    wL[:, L_BETA:L_BETA + 4] = np.asarray(wts["bn_beta"], np.float32).reshape(CC, 128).T
    wL[:, L_OAM1:L_OAM1 + 4] = (np.asarray(wts["out_a"], np.float32) - 1.0).reshape(CC, 128).T
    wL[:, L_EPS] = EPS
    wL[0:K, L_ONE191] = 1.0
    wL[0, L_ONE119:L_ONE119 + K] = 1.0

    return {
        "wpackE": wE, "wpackL": wL,
        "w2T": np.ascontiguousarray(np.asarray(wts["gcn_w2"], np.float32).T),
        "kwT": np.ascontiguousarray(np.asarray(wts["k_w"], np.float32).T),
        "vwT": np.ascontiguousarray(np.asarray(wts["v_w"], np.float32).T),
        "qwT": np.ascontiguousarray(np.asarray(wts["q_w"], np.float32).T),
        "outwT": np.ascontiguousarray(np.asarray(wts["out_w"], np.float32).T),
    }


WEIGHT_SPECS = [
    ("wpackE", [128, NE]), ("wpackL", [128, NL]),
    ("w2T", [C, C]), ("kwT", [C, CI]), ("vwT", [C, CI]),
    ("qwT", [C, CI]), ("outwT", [CI, C]),
]


def _load_chunked(nc, pool, ap, r, cdim, name):
    """DRAM [r, cdim] (r = n*128) -> SBUF [128, n*cdim], column-grouped."""
    nchunk = r // 128
    t = pool.tile([128, nchunk * cdim], F32, name=name)
    src = ap.rearrange("(n p) c -> p n c", p=128)
    nc.sync.dma_start(t[:].rearrange("p (n c) -> p n c", n=nchunk), src)
    return t


def build_caam(tc, outs, ins, n_cores, collective=True):
    nc = tc.nc
    x_d = ins["x"]
    y_d = outs["y"]
    Ntot = float(n_cores * HWp)

    # ---------------- pool stack (LIFO) ----------------
    wpoolL = tc.alloc_tile_pool(name="wtsL", bufs=1)
    dpool = tc.alloc_tile_pool(name="phD", bufs=1)
    attw = tc.alloc_tile_pool(name="attw", bufs=1)
    xpool = tc.alloc_tile_pool(name="x_res", bufs=1)
    gpool = tc.alloc_tile_pool(name="gcn", bufs=1)
    wpoolE = tc.alloc_tile_pool(name="wtsE", bufs=1)

    wE = wpoolE.tile([128, NE], F32, name="wpackE")
    nc.sync.dma_start(wE[:], ins["wpackE"])
    kwT = _load_chunked(nc, wpoolE, ins["kwT"], C, CI, "kwT")
    vwT = _load_chunked(nc, wpoolE, ins["vwT"], C, CI, "vwT")
    wL = wpoolL.tile([128, NL], F32, name="wpackL")
    nc.sync.dma_start(wL[:], ins["wpackL"])
    qwT = _load_chunked(nc, wpoolL, ins["qwT"], C, CI, "qwT")
    outwT = _load_chunked(nc, wpoolL, ins["outwT"], CI, C, "outwT")

    idn = wE[:, E_IDN:E_IDN + 128]

    # ---------------- phase A ----------------
    # x is loaded BIN-BLOCKED: free index = n*1024 + ph*32 + pw  (n = bi*4+bj)
    x_sb = []
    xv = x_d.rearrange("c h w -> c (h w)")
    xb = x_d.rearrange("c (bi h) (bj w) -> c bi bj h w", bi=BH, bj=BW)
    for cc in range(CC):
        t = xpool.tile([128, HWp], F32, name=f"x_{cc}")
        for n in range(NB):
            bi, bj = n // BW, n % BW
            nc.sync.dma_start(t[:, n * P:(n + 1) * P],
                              xb[cc * 128:(cc + 1) * 128, bi, bj])
        x_sb.append(t)

    camE = dpool.tile([K, HWp], F32, tag="bigE", name="camE")
    dsA = dpool.tile([128, NA], F32, name="dsmallA")
    dsD = dpool.tile([128, ND], F32, name="dsmallDE")
    with tc.tile_pool(name="phA_ps", bufs=1, space="PSUM") as aps:
        for nchunk in range(HWp // 512):
            cp = aps.tile([K, 512], F32, tag="camps", bufs=2)
            for cc in range(CC):
                nc.tensor.matmul(cp[:], wE[:, E_WCAM + cc*K:E_WCAM + (cc+1)*K],
                                 x_sb[cc][:, nchunk * 512:(nchunk + 1) * 512],
                                 start=(cc == 0), stop=(cc == CC - 1))
            nc.scalar.activation(camE[:, nchunk * 512:(nchunk + 1) * 512], cp[:],
                                 ACT.Identity, bias=wE[0:K, E_CAMB:E_CAMB + 1])
    for n in range(NB):
        sl = camE[0:K, n * P:(n + 1) * P]
        nc.vector.tensor_reduce(dsA[0:K, A_CSUM + n:A_CSUM + n + 1], sl, axis=AX.X, op=OP.add)
    nc.scalar.activation(camE[:], camE[:], ACT.Exp)
    for n in range(NB):
        sl = camE[0:K, n * P:(n + 1) * P]
        nc.vector.tensor_reduce(dsA[0:K, A_ESUM + n:A_ESUM + n + 1], sl, axis=AX.X, op=OP.add)
    nc.scalar.activation(dsA[0:K, A_CLS:A_CLS + NB], dsA[0:K, A_CSUM:A_CSUM + NB],
                         ACT.Sigmoid, scale=1.0 / P)
    nc.vector.reciprocal(dsA[0:K, A_REC:A_REC + NB], dsA[0:K, A_ESUM:A_ESUM + NB])
    nc.vector.tensor_mul(dsA[0:K, A_SCALE:A_SCALE + NB],
                         dsA[0:K, A_CLS:A_CLS + NB], dsA[0:K, A_REC:A_REC + NB])

    # ---------------- phase B: per-bin local ----------------
    # stack rows: p = n*19 + k; chunk0 rows 0:128 cols 0:512, chunk1 rows 0:24 cols 512:1024
    stack = gpool.tile([128, 2 * C], F32, name="stack")
    stackA = stack[:, 0:C]
    stackB = stack[0:24, C:2 * C]
    with tc.tile_pool(name="phB_sb", bufs=1) as bsb, \
         tc.tile_pool(name="phB_ps", bufs=1, space="PSUM") as bps:
        for n in range(NB):
            ET = bsb.tile([128, NB * K], F32, tag="ET")
            locp = bps.tile([K, C], F32, tag="locp", bufs=2)
            for pc in range(8):
                p0 = n * P + pc * 128
                esl = camE[0:K, p0:p0 + 128]
                etp = bps.tile([128, K], F32, tag="etp", bufs=2)
                nc.tensor.transpose(etp[:], esl, idn[0:K, 0:K])
                nc.scalar.copy(ET[:, pc * K:(pc + 1) * K], etp[:])
                xpp = bsb.tile([128, C], F32, tag="xpp", bufs=2)
                for cc in range(CC):
                    xsl = x_sb[cc][:, p0:p0 + 128]
                    xtp = bps.tile([128, 128], F32, tag="xtp", bufs=3)
                    nc.tensor.transpose(xtp[:], xsl, idn)
                    if cc % 2 == 0:
                        nc.scalar.copy(xpp[:, cc * 128:(cc + 1) * 128], xtp[:])
                    else:
                        nc.vector.tensor_copy(xpp[:, cc * 128:(cc + 1) * 128], xtp[:])
                nc.tensor.matmul(locp[:], ET[:, pc * K:(pc + 1) * K], xpp[:],
                                 start=(pc == 0), stop=(pc == 7))
            locS = bsb.tile([K, C], F32, tag="locS")
            nc.vector.tensor_single_scalar(locS[:], locp[:],
                                           dsA[0:K, A_SCALE + n:A_SCALE + n + 1], OP.mult)
            # stack rows n*19 .. n*19+19 (contiguous partitions; may straddle the
            # two chunks at p=128, i.e. bin 6: rows 114..133)
            p0 = n * K
            p1 = p0 + K
            if p1 <= 128:
                nc.sync.dma_start(stackA[p0:p1, :], locS[:, :])
            elif p0 >= 128:
                nc.sync.dma_start(stackB[p0 - 128:p1 - 128, :], locS[:, :])
            else:
                nc.sync.dma_start(stackA[p0:128, :], locS[0:128 - p0, :])
                nc.sync.dma_start(stackB[0:p1 - 128, :], locS[128 - p0:K, :])

    # ---------------- phase C: GCN + fuse + key/val ----------------
    atp = attw.tile([128, NAT], F32, name="attpack")
    keyT = atp[:, AT_KEYT:AT_KEYT + IC * KN]
    val = atp[0:K, AT_VAL:AT_VAL + CI]
    scrC = dpool.tile([128, NS], F32, tag="bigE", name="scrC")
    vA = scrC[:, S_VA:S_VA + C]
    vB = scrC[0:24, S_VB:S_VB + C]
    with tc.tile_pool(name="phC_sb", bufs=1) as csb, \
         tc.tile_pool(name="phC_ps", bufs=1, space="PSUM") as cps:
        w2T = _load_chunked(nc, csb, ins["w2T"], C, C, "w2T")
        # conv1: t = W1NK.T @ stack  (contraction over 152 stack rows, 2 chunks)
        tpA = cps.tile([128, C], F32, tag="big")
        nc.tensor.matmul(tpA[:], wE[:, E_W1NK0:E_W1NK0 + 128], stackA, start=True, stop=False)
        nc.tensor.matmul(tpA[:], wE[0:24, E_W1NK1:E_W1NK1 + 128], stackB, start=False, stop=True)
        tpB = cps.tile([24, C], F32, tag="smallB")
        nc.tensor.matmul(tpB[:], wE[:, E_W1NK0 + 128:E_W1NK0 + KN], stackA, start=True, stop=False)
        nc.tensor.matmul(tpB[:], wE[0:24, E_W1NK1 + 128:E_W1NK1 + KN], stackB, start=False, stop=True)
        # prelu(t + stack) with per-row alpha = gcn_a[n] (E_GANK cols)
        for (tp, st, vv, gchunk, rows) in ((tpA, stackA, vA, 0, 128),
                                           (tpB, stackB, vB, 1, 24)):
            u_ = scrC[0:rows, S_UG:S_UG + C]
            nc.vector.tensor_add(u_, tp[:], st)
            m_ = scrC[0:rows, S_MG:S_MG + C]
            nc.vector.tensor_scalar_min(m_, u_, 0.0)
            nc.vector.scalar_tensor_tensor(vv, m_, wE[0:rows, E_GANK + gchunk:E_GANK + gchunk + 1],
                                           u_, OP.mult, OP.add)
        # transpose t -> tT [c, (n,k)]
        for cc in range(CC):
            tt = scrC[:, S_TT + cc * KN:S_TT + (cc + 1) * KN]
            pA = cps.tile([128, 128], F32, tag="trA")
            nc.tensor.transpose(pA[:], vA[:, cc * 128:(cc + 1) * 128], idn)
            nc.scalar.copy(tt[:, 0:128], pA[:])
            pB = cps.tile([128, 24], F32, tag="trB")
            nc.tensor.transpose(pB[:], vB[:, cc * 128:(cc + 1) * 128], idn[0:24, 0:24])
            nc.scalar.copy(tt[:, 128:152], pB[:])
        # w2: local2 = t @ w2T (stack layout out)
        l2A = scrC[:, S_L2A:S_L2A + C]
        l2B = scrC[0:24, S_L2B:S_L2B + C]
        pl2A = cps.tile([128, C], F32, tag="big")
        for cc in range(CC):
            nc.tensor.matmul(pl2A[:], scrC[:, S_TT + cc * KN:S_TT + cc * KN + 128],
                             w2T[:, cc * C:(cc + 1) * C], start=(cc == 0), stop=(cc == CC - 1))
        nc.scalar.copy(l2A, pl2A[:])
        pl2B = cps.tile([24, C], F32, tag="smallB")
        for cc in range(CC):
            nc.tensor.matmul(pl2B[:], scrC[:, S_TT + cc * KN + 128:S_TT + cc * KN + 152],
                             w2T[:, cc * C:(cc + 1) * C], start=(cc == 0), stop=(cc == CC - 1))
        nc.scalar.copy(l2B, pl2B[:])
        # fuse -> glob [19, 512] (one psum tile), then prelu
        gp = cps.tile([K, C], F32, tag="gAB")
        nc.tensor.matmul(gp[:], wE[:, E_FNK0:E_FNK0 + K], l2A, start=True, stop=False)
        nc.tensor.matmul(gp[:], wE[0:24, E_FNK1:E_FNK1 + K], l2B, start=False, stop=True)
        glob = scrC[0:K, S_GL:S_GL + C]
        u_ = scrC[0:K, S_UG2:S_UG2 + C]
        nc.vector.tensor_scalar_add(u_, gp[:], wE[0:K, E_FB:E_FB + 1])
        m_ = scrC[0:K, S_MG2:S_MG2 + C]
        nc.vector.tensor_scalar_min(m_, u_, 0.0)
        nc.vector.scalar_tensor_tensor(glob, m_, wE[0:K, E_RAM1:E_RAM1 + 1], u_, OP.mult, OP.add)
        # globT + val (+ v_b via ones-row matmul)
        valp = cps.tile([K, CI], F32, tag="valp")
        for cc in range(CC):
            gt = scrC[:, S_GT + cc * K:S_GT + (cc + 1) * K]
            pA = cps.tile([128, K], F32, tag="trB")
            nc.tensor.transpose(pA[:], glob[:, cc * 128:(cc + 1) * 128], idn[0:K, 0:K])
            nc.scalar.copy(gt[:, :], pA[:])
            nc.tensor.matmul(valp[:], gt[:], vwT[:, cc * CI:(cc + 1) * CI],
                             start=(cc == 0), stop=False)
        nc.tensor.matmul(valp[:], wE[0:1, E_ONE119:E_ONE119 + K], wE[0:1, E_VB:E_VB + CI],
                         start=False, stop=True)
        nc.scalar.copy(val, valp[:])
        # local2T + keyT (+ k_b per-partition bias)
        for cc in range(CC):
            lt = scrC[:, S_L2T + cc * KN:S_L2T + (cc + 1) * KN]
            pA = cps.tile([128, 128], F32, tag="trA")
            nc.tensor.transpose(pA[:], l2A[:, cc * 128:(cc + 1) * 128], idn)
            nc.scalar.copy(lt[:, 0:128], pA[:])
            pB = cps.tile([128, 24], F32, tag="trB")
            nc.tensor.transpose(pB[:], l2B[:, cc * 128:(cc + 1) * 128], idn[0:24, 0:24])
            nc.scalar.copy(lt[:, 128:152], pB[:])
        for ic in range(IC):
            kp = cps.tile([128, KN], F32, tag="keyp")
            for cc in range(CC):
                nc.tensor.matmul(kp[:], kwT[:, cc * CI + ic * 128: cc * CI + (ic + 1) * 128],
                                 scrC[:, S_L2T + cc * KN:S_L2T + (cc + 1) * KN],
                                 start=(cc == 0), stop=(cc == CC - 1))
            nc.scalar.activation(keyT[:, ic * KN:(ic + 1) * KN], kp[:], ACT.Identity,
                                 bias=wE[:, E_KB + ic:E_KB + ic + 1])
    wpoolE.release()
    gpool.release()

    # ---------------- phase D+E: attention + y stats ----------------
    Eall = dpool.tile([K, HWp], F32, tag="bigE", name="Eall")
    with tc.tile_pool(name="phD_sb", bufs=1) as dsb, \
         tc.tile_pool(name="phD_ps", bufs=1, space="PSUM") as dps:
        for n in range(NB):
            qT = dsb.tile([128, IC * P], F32, tag="qT")
            for ic in range(IC):
                for nh in range(2):
                    qp = dps.tile([128, 512], F32, tag="qp")
                    for cc in range(CC):
                        xsl = x_sb[cc][:, n * P + nh * 512: n * P + (nh + 1) * 512]
                        nc.tensor.matmul(qp[:], qwT[:, cc * CI + ic * 128: cc * CI + (ic + 1) * 128],
                                         xsl, start=(cc == 0), stop=(cc == CC - 1))
                    nc.scalar.activation(qT[:, ic * P + nh * 512: ic * P + (nh + 1) * 512], qp[:],
                                         ACT.Identity, bias=wL[:, L_QB + ic:L_QB + ic + 1])
            ebin = Eall[:, n * P:(n + 1) * P]
            for nh in range(2):
                afp = dps.tile([K, 512], F32, tag="afp")
                for ic in range(IC):
                    ksel = keyT[:, ic * KN + n * K: ic * KN + (n + 1) * K]
                    nc.tensor.matmul(afp[:], ksel, qT[:, ic * P + nh * 512: ic * P + (nh + 1) * 512],
                                     start=(ic == 0), stop=(ic == IC - 1))
                nc.scalar.activation(ebin[:, nh * 512:(nh + 1) * 512], afp[:], ACT.Exp)
                sp = dps.tile([1, 512], F32, tag="sp")
                nc.tensor.matmul(sp[:], wL[0:K, L_ONE191:L_ONE191 + 1],
                                 ebin[:, nh * 512:(nh + 1) * 512], start=True, stop=True)
                rrow = dsD[0:1, D_RROW:D_RROW + 512]
                nc.vector.reciprocal(rrow, sp[:])
                rbp = dps.tile([K, 512], F32, tag="rbp")
                nc.tensor.matmul(rbp[:], wL[0:1, L_ONE119:L_ONE119 + K], rrow, start=True, stop=True)
                nc.vector.tensor_mul(ebin[:, nh * 512:(nh + 1) * 512],
                                     ebin[:, nh * 512:(nh + 1) * 512], rbp[:])
            attnT = []
            for ic in range(IC):
                aop = dps.tile([128, P], F32, tag="aop")
                for nh in range(2):
                    nc.tensor.matmul(aop[:, nh * 512:(nh + 1) * 512], val[:, ic * 128:(ic + 1) * 128],
                                     ebin[:, nh * 512:(nh + 1) * 512], start=True, stop=True)
                at = dsb.tile([128, P], F32, tag="attnT", bufs=2)
                nc.scalar.activation(at[:], aop[:], ACT.Copy,
                                     accum_out=dsD[:, D_RS + ic * NB + n: D_RS + ic * NB + n + 1])
                attnT.append(at)
            for cc in range(CC):
                for nh in range(2):
                    yp = dps.tile([128, 512], F32, tag="yp", bufs=2)
                    for ic in range(IC):
                        nc.tensor.matmul(yp[:], outwT[:, ic * C + cc * 128: ic * C + (cc + 1) * 128],
                                         attnT[ic][:, nh * 512:(nh + 1) * 512],
                                         start=(ic == 0), stop=(ic == IC - 1))
                    ysq = dsb.tile([128, 512], F32, tag="ysq", bufs=2)
                    col = D_SQ + cc * 2 * NB + n * 2 + nh
                    nc.scalar.activation(ysq[:], yp[:], ACT.Square, accum_out=dsD[:, col:col + 1])
        for ic in range(IC):
            nc.vector.tensor_reduce(dsD[:, D_RSUM + ic:D_RSUM + ic + 1],
                                    dsD[:, D_RS + ic * NB:D_RS + (ic + 1) * NB], axis=AX.X, op=OP.add)
        for cc in range(CC):
            mup = dps.tile([128, 1], F32, tag="yp", bufs=2)
            for ic in range(IC):
                nc.tensor.matmul(mup[:], outwT[:, ic * C + cc * 128: ic * C + (cc + 1) * 128],
                                 dsD[:, D_RSUM + ic:D_RSUM + ic + 1], start=(ic == 0), stop=(ic == IC - 1))
            nc.vector.tensor_copy(dsD[:, D_ST + 2 * cc:D_ST + 2 * cc + 1], mup[:])
            nc.vector.tensor_reduce(dsD[:, D_ST + 2 * cc + 1:D_ST + 2 * cc + 2],
                                    dsD[:, D_SQ + cc * 2 * NB:D_SQ + (cc + 1) * 2 * NB],
                                    axis=AX.X, op=OP.add)
    xpool.release()

    # ---------------- collective ----------------
    with tc.tile_pool(name="cdram", bufs=1, space="DRAM") as cdram:
        arin = cdram.tile([128, 2 * CC], F32)
        arout = cdram.tile([128, 2 * CC], F32)
        nc.sync.dma_start(arin[:], dsD[:, D_ST:D_ST + 2 * CC])
        if collective:
            nc.gpsimd.collective_compute(
                "AllReduce", OP.add,
                ins=[arin.opt()], outs=[arout.opt()],
                replica_groups=[list(range(n_cores))],
            )
            nc.sync.dma_start(dsD[:, D_SBN:D_SBN + 2 * CC], arout[:])
        else:
            nc.sync.dma_start(dsD[:, D_SBN:D_SBN + 2 * CC], arin[:])

    # ---------------- BN finalize ----------------
    mom = dsD[:, D_MOM:D_MOM + 2 * CC]
    nc.scalar.mul(mom, dsD[:, D_SBN:D_SBN + 2 * CC], 1.0 / Ntot)
    muv = mom.rearrange("p (c two) -> p c two", two=2)[:, :, 0]
    msq = mom.rearrange("p (c two) -> p c two", two=2)[:, :, 1]
    nc.vector.tensor_mul(dsD[:, D_MUSQ:D_MUSQ + CC], muv, muv)
    nc.vector.tensor_sub(dsD[:, D_VAR:D_VAR + CC], msq, dsD[:, D_MUSQ:D_MUSQ + CC])
    nc.scalar.activation(dsD[:, D_SD:D_SD + CC], dsD[:, D_VAR:D_VAR + CC], ACT.Sqrt,
                         bias=wL[:, L_EPS:L_EPS + 1])
    nc.vector.reciprocal(dsD[:, D_RSTD:D_RSTD + CC], dsD[:, D_SD:D_SD + CC])
    scol = dsD[:, D_SCOL:D_SCOL + CC]
    bcol = dsD[:, D_BCOL:D_BCOL + CC]
    nc.vector.tensor_mul(scol, wL[:, L_GAMMA:L_GAMMA + CC], dsD[:, D_RSTD:D_RSTD + CC])
    nc.vector.tensor_scalar_mul(dsD[:, D_NSC:D_NSC + CC], scol, -1.0)
    for cc in range(CC):
        nc.vector.scalar_tensor_tensor(bcol[:, cc:cc + 1], muv[:, cc:cc + 1],
                                       dsD[:, D_NSC + cc:D_NSC + cc + 1],
                                       wL[:, L_BETA + cc:L_BETA + cc + 1], OP.mult, OP.add)

    # ---------------- phase F ----------------
    yv = y_d.rearrange("c h w -> c (h w)")
    with tc.tile_pool(name="phF_sb", bufs=1) as fsb, \
         tc.tile_pool(name="phF_ps", bufs=1, space="PSUM") as fps:
        for bi in range(BH):
            stage = []
            xbr = []
            for cc in range(CC):
                st_ = fsb.tile([128, RH * W], F32, tag=f"stage{cc}")
                stage.append(st_)
                xb = fsb.tile([128, RH * W], F32, tag=f"xbr{cc}")
                nc.sync.dma_start(xb[:], xv[cc * 128:(cc + 1) * 128, RH * bi * W:RH * (bi + 1) * W])
                xbr.append(xb)
            for bj in range(BW):
                n = bi * BW + bj
                ebin = Eall[:, n * P:(n + 1) * P]
                attnT = []
                for ic in range(IC):
                    aop = fps.tile([128, P], F32, tag="aop2")
                    for nh in range(2):
                        nc.tensor.matmul(aop[:, nh * 512:(nh + 1) * 512], val[:, ic * 128:(ic + 1) * 128],
                                         ebin[:, nh * 512:(nh + 1) * 512], start=True, stop=True)
                    at = fsb.tile([128, P], F32, tag="attnT2", bufs=2)
                    nc.scalar.copy(at[:], aop[:])
                    attnT.append(at)
                for cc in range(CC):
                    for nh in range(2):
                        yp = fps.tile([128, 512], F32, tag="yp2", bufs=2)
                        for ic in range(IC):
                            nc.tensor.matmul(yp[:], outwT[:, ic * C + cc * 128: ic * C + (cc + 1) * 128],
                                             attnT[ic][:, nh * 512:(nh + 1) * 512],
                                             start=(ic == 0), stop=(ic == IC - 1))
                        u = fsb.tile([128, 512], F32, tag="u_f", bufs=2)
                        nc.scalar.activation(u[:], yp[:], ACT.Identity,
                                             bias=bcol[:, cc:cc + 1], scale=scol[:, cc:cc + 1])
                        m = fsb.tile([128, 512], F32, tag="m_f", bufs=2)
                        nc.gpsimd.tensor_scalar_min(m[:], u[:], 0.0)
                        v = fsb.tile([128, 512], F32, tag="v_f", bufs=2)
                        nc.vector.scalar_tensor_tensor(v[:], m[:], wL[:, L_OAM1 + cc:L_OAM1 + cc + 1],
                                                       u[:], OP.mult, OP.add)
                        dst = stage[cc][:].rearrange("p (h w) -> p h w", w=W)[
                            :, 16 * nh:16 * (nh + 1), RW * bj:RW * (bj + 1)]
                        xres = xbr[cc][:].rearrange("p (h w) -> p h w", w=W)[
                            :, 16 * nh:16 * (nh + 1), RW * bj:RW * (bj + 1)]
                        nc.vector.tensor_add(dst, v[:], xres)
            for cc in range(CC):
                nc.sync.dma_start(yv[cc * 128:(cc + 1) * 128, RH * bi * W:RH * (bi + 1) * W], stage[cc][:])
    attw.release()
    dpool.release()
    wpoolL.release()


# ======================================================================
# Entry point: kernel(**inputs) -> np.ndarray [8, 512, 64, 128]
# ======================================================================
import concourse.bacc as bacc
import concourse.tile as tile
from concourse.bass_utils import run_bass_kernel_spmd

N_CORES = 8
_cached = {}


def _build_program(n_cores=N_CORES):
    if "nc" in _cached:
        return _cached["nc"]
    nc = bacc.Bacc("TRN2", target_bir_lowering=False, debug=False, num_devices=n_cores)
    ins = {"x": nc.dram_tensor("x", [C, H, W], F32, kind="ExternalInput").ap()}
    for nm, shape in WEIGHT_SPECS:
        ins[nm] = nc.dram_tensor(nm, shape, F32, kind="ExternalInput").ap()
    outs = {"y": nc.dram_tensor("y", [C, H, W], F32, kind="ExternalOutput").ap()}
    with tile.TileContext(nc) as tc:
        build_caam(tc, outs, ins, n_cores)
    nc.compile()
    _cached["nc"] = nc
    return nc


def make_in_maps(inputs):
    x = np.ascontiguousarray(np.asarray(inputs["x"], np.float32))
    prep = host_prep(inputs)
    in_maps = []
    for c in range(N_CORES):
        d = {"x": np.ascontiguousarray(x[c])}
        for nm, _ in WEIGHT_SPECS:
            d[nm] = prep[nm]
        in_maps.append(d)
    return in_maps


def kernel(**inputs):
    nc = _build_program()
    in_maps = make_in_maps(inputs)
    res = run_bass_kernel_spmd(nc, in_maps, core_ids=list(range(N_CORES)))
    return np.stack([res.results[c]["y"] for c in range(N_CORES)]).astype(np.float32)



# revision 12
# speedup vs baseline: 7.3897x; 7.3897x over previous
"""CAAM kernel for Trainium2: builder + host-side prep.

Per-core: one batch element, everything bf16 on the PE path (1 cy/row vs 4
for fp32), fp32 accumulation in PSUM and fp32 BN statistics.

Layout: x and y resident in SBUF as bf16 [128, 8192] BIN-BLOCKED
(p = n*1024 + hh*32 + ww; host pre-permutes x and inverse-permutes y), so
every matmul moving operand is a dense 1-free-dim slice (HW requirement)
and all DMAs are contiguous. Single pass: y is produced once (phase D),
kept resident, and phase F only applies the BN affine + PReLU + residual
(no recompute, no second x load). Output DRAM tensor is bf16; host casts
to fp32.
"""

import numpy as np
import ml_dtypes
import concourse.bass as bass
import concourse.mybir as mybir

F32 = mybir.dt.float32
BF16 = mybir.dt.bfloat16
NPBF = ml_dtypes.bfloat16
AX = mybir.AxisListType
OP = mybir.AluOpType
ACT = mybir.ActivationFunctionType

B, C, H, W = 8, 512, 64, 128
K, BH, BW = 19, 2, 4
NB = BH * BW          # 8
CI = C // 2           # 256
HWp = H * W           # 8192
RH, RW = H // BH, W // BW   # 32, 32
P = RH * RW           # 1024
CC = C // 128         # 4
IC = CI // 128        # 2
KN = K * NB           # 152
EPS = 1e-5

# -------- wpackB column map (bf16 consts) --------
BW_IDN = 0          # 128
BW_WCAM = 128       # 4*19 = 76
BW_W1NK0 = 204      # 152
BW_W1NK1 = 356      # 152 (rows 0:24)
BW_FNK0 = 508       # 19
BW_FNK1 = 527       # 19 (rows 0:24)
BW_ONES19 = 546     # 1 col, rows 0:19 (ones; sp rowsum lhsT)
BW_ONER19 = 547     # 19 cols, row 0 (ones; rbp/vb lhsT)
BW_VB = 566         # 256 cols, row 0 (v_b)
BW_OAM1 = 822       # 4 cols (out_a - 1)
NBW = 826

# -------- wpackF column map (fp32 consts) --------
F_CAMB = 0          # 1 col rows 0:19
F_GANK = 1          # 2 cols (gcn_a - 1 per stack-row chunk)
F_FB = 3            # 1 col rows 0:19
F_RAM1 = 4          # 1 col rows 0:19
F_KB = 5            # 2 cols
F_QB = 7            # 2 cols
F_GAMMA = 9         # 4
F_BETA = 13         # 4
F_EPS = 17          # 1
NF = 18

# -------- dsmall (fp32 stats) column map --------
D_CS = 0            # 16 (cam sums per (bin, nh))
D_ES = 16           # 16 (exp sums per (bin, nh))
D_CSB = 32          # 8
D_ESB = 40          # 8
D_CLS = 48          # 8
D_REC = 56          # 8
D_SCALE = 64        # 8
D_RS = 72           # 32 (attnT row sums per (ic, bin, nh))
D_SQ = 104          # 64 (y^2 sums per (cc, bin, nh))
D_RSUM = 168        # 2
D_ST = 170          # 8 (packed allreduce input)
D_SBN = 178         # 8 (allreduce output)
D_MOM = 186         # 8
D_VAR = 194         # 4
D_MUSQ = 198        # 4
D_SD = 202          # 4
D_RSTD = 206        # 4
D_SCOL = 210        # 4
D_NSC = 214         # 4
D_BCOL = 218        # 4
ND = 222


def host_prep(wts: dict) -> dict:
    w1 = np.asarray(wts["gcn_w1"], np.float32)
    ga = np.asarray(wts["gcn_a"], np.float32)
    fw = np.asarray(wts["fuse_w"], np.float32).reshape(-1)
    fb = float(np.asarray(wts["fuse_b"], np.float32).reshape(-1)[0])
    ra = float(np.asarray(wts["relu_a"], np.float32).reshape(-1)[0])

    wB = np.zeros((128, NBW), np.float32)
    W1NK = np.zeros((KN, KN), np.float32)
    FNK = np.zeros((KN, K), np.float32)
    ga_nk = np.zeros(KN, np.float32)
    for n in range(NB):
        for k in range(K):
            for m in range(NB):
                W1NK[m*K + k, n*K + k] = w1[n, m]
            FNK[n*K + k, k] = fw[n]
            ga_nk[n*K + k] = ga[n] - 1.0
    wB[:, BW_IDN:BW_IDN + 128] = np.eye(128, dtype=np.float32)
    wcamT = np.asarray(wts["conv_cam_w"], np.float32).T    # [512, 19]
    for cc in range(CC):
        wB[:, BW_WCAM + cc*K:BW_WCAM + (cc+1)*K] = wcamT[cc*128:(cc+1)*128]
    wB[:, BW_W1NK0:BW_W1NK0 + KN] = W1NK[0:128]
    wB[0:24, BW_W1NK1:BW_W1NK1 + KN] = W1NK[128:KN]
    wB[:, BW_FNK0:BW_FNK0 + K] = FNK[0:128]
    wB[0:24, BW_FNK1:BW_FNK1 + K] = FNK[128:KN]
    wB[0:K, BW_ONES19] = 1.0
    wB[0, BW_ONER19:BW_ONER19 + K] = 1.0
    wB[0, BW_VB:BW_VB + CI] = np.asarray(wts["v_b"], np.float32)
    wB[:, BW_OAM1:BW_OAM1 + CC] = (np.asarray(wts["out_a"], np.float32) - 1.0).reshape(CC, 128).T

    wF = np.zeros((128, NF), np.float32)
    wF[0:K, F_CAMB] = np.asarray(wts["conv_cam_b"], np.float32)
    wF[:, F_GANK] = ga_nk[0:128]
    wF[0:24, F_GANK + 1] = ga_nk[128:KN]
    wF[0:K, F_FB] = fb
    wF[0:K, F_RAM1] = ra - 1.0
    wF[:, F_KB:F_KB + IC] = np.asarray(wts["k_b"], np.float32).reshape(IC, 128).T
    wF[:, F_QB:F_QB + IC] = np.asarray(wts["q_b"], np.float32).reshape(IC, 128).T
    wF[:, F_GAMMA:F_GAMMA + CC] = np.asarray(wts["bn_gamma"], np.float32).reshape(CC, 128).T
    wF[:, F_BETA:F_BETA + CC] = np.asarray(wts["bn_beta"], np.float32).reshape(CC, 128).T
    wF[:, F_EPS] = EPS

    def bf(a):
        return np.ascontiguousarray(np.asarray(a, np.float32)).astype(NPBF)

    return {
        "wpackB": wB.astype(NPBF), "wpackF": wF,
        "w2T": bf(np.asarray(wts["gcn_w2"], np.float32).T),
        "kwT": bf(np.asarray(wts["k_w"], np.float32).T),
        "vwT": bf(np.asarray(wts["v_w"], np.float32).T),
        "qwT": bf(np.asarray(wts["q_w"], np.float32).T),
        "outwT": bf(np.asarray(wts["out_w"], np.float32).T),
    }


WEIGHT_SPECS = [
    ("wpackB", [128, NBW], BF16), ("wpackF", [128, NF], F32),
    ("w2T", [C, C], BF16), ("kwT", [C, CI], BF16), ("vwT", [C, CI], BF16),
    ("qwT", [C, CI], BF16), ("outwT", [CI, C], BF16),
]


def _load_chunked(nc, pool, ap, r, cdim, name, dt=BF16):
    """DRAM [r, cdim] (r = n*128) -> SBUF [128, n*cdim], column-grouped."""
    nchunk = r // 128
    t = pool.tile([128, nchunk * cdim], dt, name=name)
    src = ap.rearrange("(n p) c -> p n c", p=128)
    nc.sync.dma_start(t[:].rearrange("p (n c) -> p n c", n=nchunk), src)
    return t


def build_caam(tc, outs, ins, n_cores, collective=True):
    nc = tc.nc
    with nc.allow_low_precision(reason="bf16 kernel; rel-err budget 2e-2"):
        _build_caam(tc, outs, ins, n_cores, collective)


def _build_caam(tc, outs, ins, n_cores, collective):
    nc = tc.nc
    x_d = ins["x"]
    y_d = outs["y"]
    Ntot = float(n_cores * HWp)

    # ---------------- persistent pools ----------------
    wpool = tc.alloc_tile_pool(name="wts", bufs=1)
    xpool = tc.alloc_tile_pool(name="x_res", bufs=1)
    ypool = tc.alloc_tile_pool(name="y_res", bufs=1)
    spool = tc.alloc_tile_pool(name="smalls", bufs=1)

    wB = wpool.tile([128, NBW], BF16, name="wpackB")
    nc.sync.dma_start(wB[:], ins["wpackB"])
    wF = wpool.tile([128, NF], F32, name="wpackF")
    nc.sync.dma_start(wF[:], ins["wpackF"])
    kwT = _load_chunked(nc, wpool, ins["kwT"], C, CI, "kwT")
    vwT = _load_chunked(nc, wpool, ins["vwT"], C, CI, "vwT")
    qwT = _load_chunked(nc, wpool, ins["qwT"], C, CI, "qwT")
    outwT = _load_chunked(nc, wpool, ins["outwT"], CI, C, "outwT")

    idn = wB[:, BW_IDN:BW_IDN + 128]

    dsm = spool.tile([128, ND], F32, name="dsmall")
    keyT = spool.tile([128, IC * KN], BF16, name="keyT")
    val = spool.tile([K, CI], BF16, name="val")
    rsb = spool.tile([128, IC], BF16, name="rsb")

    # x resident, bin-blocked (host pre-permuted), contiguous loads
    x_sb = []
    for cc in range(CC):
        t = xpool.tile([128, HWp], BF16, name=f"x_{cc}")
        nc.sync.dma_start(t[:], x_d[cc * 128:(cc + 1) * 128, :])
        x_sb.append(t)
    y_sb = [ypool.tile([128, HWp], BF16, name=f"y_{cc}") for cc in range(CC)]

    def xbin(cc, n, nh):
        return x_sb[cc][:, n*P + nh*512:n*P + (nh+1)*512]

    def xpc(cc, n, pc):
        return x_sb[cc][:, n*P + pc*128:n*P + (pc+1)*128]

    def ybin(cc, n, nh):
        return y_sb[cc][:, n*P + nh*512:n*P + (nh+1)*512]

    # ---------------- phase A+B fused: cam + stats + local, per bin ----------------
    stackpool = tc.alloc_tile_pool(name="stack", bufs=1)
    stack = stackpool.tile([128, 2 * C], BF16, name="stack")
    stackA = stack[:, 0:C]
    stackB = stack[0:24, C:2 * C]
    with tc.tile_pool(name="phAB_sb", bufs=1) as absb, \
         tc.tile_pool(name="phAB_ps", bufs=1, space="PSUM") as abps:
        for n in range(NB):
            ebA = absb.tile([K, P], BF16, tag="ebA", bufs=2)
            for nh in range(2):
                cp = abps.tile([K, 512], F32, tag="camps", bufs=2)
                for cc in range(CC):
                    nc.tensor.matmul(cp[:], wB[:, BW_WCAM + cc*K:BW_WCAM + (cc+1)*K],
                                     xbin(cc, n, nh), start=(cc == 0), stop=(cc == CC - 1))
                nc.vector.tensor_reduce(dsm[0:K, D_CS + n*2 + nh:D_CS + n*2 + nh + 1],
                                        cp[:], axis=AX.X, op=OP.add)
                nc.scalar.activation(ebA[:, nh*512:(nh+1)*512], cp[:], ACT.Exp,
                                     bias=wF[0:K, F_CAMB:F_CAMB + 1],
                                     accum_out=dsm[0:K, D_ES + n*2 + nh:D_ES + n*2 + nh + 1])
            nc.vector.tensor_add(dsm[0:K, D_CSB + n:D_CSB + n + 1],
                                 dsm[0:K, D_CS + n*2:D_CS + n*2 + 1],
                                 dsm[0:K, D_CS + n*2 + 1:D_CS + n*2 + 2])
            nc.vector.tensor_add(dsm[0:K, D_ESB + n:D_ESB + n + 1],
                                 dsm[0:K, D_ES + n*2:D_ES + n*2 + 1],
                                 dsm[0:K, D_ES + n*2 + 1:D_ES + n*2 + 2])
            nc.scalar.activation(dsm[0:K, D_CLS + n:D_CLS + n + 1],
                                 dsm[0:K, D_CSB + n:D_CSB + n + 1],
                                 ACT.Sigmoid, scale=1.0 / P, bias=wF[0:K, F_CAMB:F_CAMB + 1])
            nc.vector.reciprocal(dsm[0:K, D_REC + n:D_REC + n + 1],
                                 dsm[0:K, D_ESB + n:D_ESB + n + 1])
            nc.vector.tensor_mul(dsm[0:K, D_SCALE + n:D_SCALE + n + 1],
                                 dsm[0:K, D_CLS + n:D_CLS + n + 1],
                                 dsm[0:K, D_REC + n:D_REC + n + 1])
            # local = E @ x^T per bin (contract over pixels, 8 chunks of 128)
            ET = absb.tile([128, KN], BF16, tag="ET", bufs=2)
            locp = abps.tile([K, C], F32, tag="locp", bufs=2)
            for pc in range(8):
                etp = abps.tile([128, K], BF16, tag="etp", bufs=2)
                nc.tensor.transpose(etp[:], ebA[:, pc*128:(pc+1)*128], idn[0:K, 0:K])
                nc.scalar.copy(ET[:, pc*K:(pc+1)*K], etp[:])
                xpp = absb.tile([128, C], BF16, tag="xpp", bufs=3)
                for cc in range(CC):
                    xtp = abps.tile([128, 128], BF16, tag="xtp", bufs=2)
                    nc.tensor.transpose(xtp[:], xpc(cc, n, pc), idn)
                    if cc % 2 == 0:
                        nc.scalar.copy(xpp[:, cc*128:(cc+1)*128], xtp[:])
                    else:
                        nc.vector.tensor_copy(xpp[:, cc*128:(cc+1)*128], xtp[:])
                nc.tensor.matmul(locp[:], ET[:, pc*K:(pc+1)*K], xpp[:],
                                 start=(pc == 0), stop=(pc == 7))
            locS = absb.tile([K, C], BF16, tag="locS", bufs=2)
            nc.vector.tensor_single_scalar(locS[:], locp[:],
                                           dsm[0:K, D_SCALE + n:D_SCALE + n + 1], OP.mult)
            # stack rows n*19 .. n*19+19 (may straddle chunk boundary at 128)
            p0, p1 = n * K, n * K + K
            if p1 <= 128:
                nc.sync.dma_start(stackA[p0:p1, :], locS[:, :])
            elif p0 >= 128:
                nc.sync.dma_start(stackB[p0 - 128:p1 - 128, :], locS[:, :])
            else:
                nc.sync.dma_start(stackA[p0:128, :], locS[0:128 - p0, :])
                nc.sync.dma_start(stackB[0:p1 - 128, :], locS[128 - p0:K, :])

    # ---------------- phase C: GCN + fuse + key/val ----------------
    with tc.tile_pool(name="phC_sb", bufs=1) as csb, \
         tc.tile_pool(name="phC_ps", bufs=1, space="PSUM") as cps:
        w2T = _load_chunked(nc, csb, ins["w2T"], C, C, "w2T")
        # conv1: t = W1NK.T @ stack (contraction over 152 stack rows, 2 chunks)
        tpA = cps.tile([128, C], F32, tag="big")
        nc.tensor.matmul(tpA[:], wB[:, BW_W1NK0:BW_W1NK0 + 128], stackA, start=True, stop=False)
        nc.tensor.matmul(tpA[:], wB[0:24, BW_W1NK1:BW_W1NK1 + 128], stackB, start=False, stop=True)
        tpB = cps.tile([24, C], F32, tag="smallB")
        nc.tensor.matmul(tpB[:], wB[:, BW_W1NK0 + 128:BW_W1NK0 + KN], stackA, start=True, stop=False)
        nc.tensor.matmul(tpB[:], wB[0:24, BW_W1NK1 + 128:BW_W1NK1 + KN], stackB, start=False, stop=True)
        # prelu(t + stack), per-row alpha = gcn_a[n]
        vA = csb.tile([128, C], BF16, name="vA")
        vBt = csb.tile([24, C], BF16, name="vB")
        for (tp, st, vv, gchunk, rows) in ((tpA, stackA, vA[:], 0, 128),
                                           (tpB, stackB, vBt[:], 1, 24)):
            u_ = csb.tile([128, C], F32, tag="uC", bufs=2)
            nc.vector.tensor_add(u_[0:rows, :], tp[:], st)
            m_ = csb.tile([128, C], F32, tag="mC", bufs=2)
            nc.vector.tensor_scalar_min(m_[0:rows, :], u_[0:rows, :], 0.0)
            nc.vector.scalar_tensor_tensor(vv, m_[0:rows, :],
                                           wF[0:rows, F_GANK + gchunk:F_GANK + gchunk + 1],
                                           u_[0:rows, :], OP.mult, OP.add)
        # transpose t -> tT [c, (n,k)]
        tT = csb.tile([128, CC * KN], BF16, name="tT")
        for cc in range(CC):
            pA = cps.tile([128, 128], BF16, tag="trA", bufs=1)
            nc.tensor.transpose(pA[:], vA[:, cc*128:(cc+1)*128], idn)
            nc.scalar.copy(tT[:, cc*KN:cc*KN + 128], pA[:])
            pB = cps.tile([128, 24], BF16, tag="trB", bufs=1)
            nc.tensor.transpose(pB[:], vBt[:, cc*128:(cc+1)*128], idn[0:24, 0:24])
            nc.scalar.copy(tT[:, cc*KN + 128:(cc+1)*KN], pB[:])
        # w2: local2 = t @ w2T (stack layout out)
        l2A = csb.tile([128, C], BF16, name="l2A")
        l2B = csb.tile([24, C], BF16, name="l2B")
        pl2A = cps.tile([128, C], F32, tag="big")
        for cc in range(CC):
            nc.tensor.matmul(pl2A[:], tT[:, cc*KN:cc*KN + 128],
                             w2T[:, cc*C:(cc+1)*C], start=(cc == 0), stop=(cc == CC - 1))
        nc.scalar.copy(l2A[:], pl2A[:])
        pl2B = cps.tile([24, C], F32, tag="smallB")
        for cc in range(CC):
            nc.tensor.matmul(pl2B[:], tT[:, cc*KN + 128:(cc+1)*KN],
                             w2T[:, cc*C:(cc+1)*C], start=(cc == 0), stop=(cc == CC - 1))
        nc.scalar.copy(l2B[:], pl2B[:])
        # fuse -> glob [19, 512], then prelu
        gp = cps.tile([K, C], F32, tag="gAB")
        nc.tensor.matmul(gp[:], wB[:, BW_FNK0:BW_FNK0 + K], l2A[:], start=True, stop=False)
        nc.tensor.matmul(gp[:], wB[0:24, BW_FNK1:BW_FNK1 + K], l2B[:], start=False, stop=True)
        glob = csb.tile([K, C], BF16, name="glob")
        ug = csb.tile([K, C], F32, name="ug")
        nc.vector.tensor_scalar_add(ug[:], gp[:], wF[0:K, F_FB:F_FB + 1])
        mg = csb.tile([K, C], F32, name="mg")
        nc.vector.tensor_scalar_min(mg[:], ug[:], 0.0)
        nc.vector.scalar_tensor_tensor(glob[:], mg[:], wF[0:K, F_RAM1:F_RAM1 + 1],
                                       ug[:], OP.mult, OP.add)
        # globT -> val (+ v_b via ones-row matmul)
        valp = cps.tile([K, CI], F32, tag="valp")
        gT = csb.tile([128, CC * K], BF16, name="gT")
        for cc in range(CC):
            pA = cps.tile([128, K], BF16, tag="trB", bufs=1)
            nc.tensor.transpose(pA[:], glob[:, cc*128:(cc+1)*128], idn[0:K, 0:K])
            nc.scalar.copy(gT[:, cc*K:(cc+1)*K], pA[:])
            nc.tensor.matmul(valp[:], gT[:, cc*K:(cc+1)*K], vwT[:, cc*CI:(cc+1)*CI],
                             start=(cc == 0), stop=False)
        nc.tensor.matmul(valp[:], wB[0:1, BW_ONER19:BW_ONER19 + K], wB[0:1, BW_VB:BW_VB + CI],
                         start=False, stop=True)
        nc.scalar.copy(val[:], valp[:])
        # local2T -> keyT (+ k_b per-partition bias)
        l2T = csb.tile([128, CC * KN], BF16, name="l2T")
        for cc in range(CC):
            pA = cps.tile([128, 128], BF16, tag="trA", bufs=1)
            nc.tensor.transpose(pA[:], l2A[:, cc*128:(cc+1)*128], idn)
            nc.scalar.copy(l2T[:, cc*KN:cc*KN + 128], pA[:])
            pB = cps.tile([128, 24], BF16, tag="trB", bufs=1)
            nc.tensor.transpose(pB[:], l2B[:, cc*128:(cc+1)*128], idn[0:24, 0:24])
            nc.scalar.copy(l2T[:, cc*KN + 128:(cc+1)*KN], pB[:])
        for ic in range(IC):
            kp = cps.tile([128, KN], F32, tag="keyp", bufs=2)
            for cc in range(CC):
                nc.tensor.matmul(kp[:], kwT[:, cc*CI + ic*128:cc*CI + (ic+1)*128],
                                 l2T[:, cc*KN:(cc+1)*KN],
                                 start=(cc == 0), stop=(cc == CC - 1))
            nc.scalar.activation(keyT[:, ic*KN:(ic+1)*KN], kp[:], ACT.Identity,
                                 bias=wF[:, F_KB + ic:F_KB + ic + 1])
    stackpool.release()

    # ---------------- phase D: attention + y + stats ----------------
    with tc.tile_pool(name="phD_sb", bufs=1) as dsb, \
         tc.tile_pool(name="phD_ps", bufs=1, space="PSUM") as dps:
        for n in range(NB):
            qT = dsb.tile([128, IC * P], BF16, tag="qT", bufs=2)
            for ic in range(IC):
                for nh in range(2):
                    qp = dps.tile([128, 512], F32, tag="qp", bufs=2)
                    for cc in range(CC):
                        nc.tensor.matmul(qp[:], qwT[:, cc*CI + ic*128:cc*CI + (ic+1)*128],
                                         xbin(cc, n, nh), start=(cc == 0), stop=(cc == CC - 1))
                    nc.scalar.activation(qT[:, ic*P + nh*512:ic*P + (nh+1)*512], qp[:],
                                         ACT.Identity, bias=wF[:, F_QB + ic:F_QB + ic + 1])
            eb = dsb.tile([K, P], BF16, tag="ebD", bufs=2)
            for nh in range(2):
                afp = dps.tile([K, 512], F32, tag="afp", bufs=1)
                for ic in range(IC):
                    nc.tensor.matmul(afp[:], keyT[:, ic*KN + n*K:ic*KN + (n+1)*K],
                                     qT[:, ic*P + nh*512:ic*P + (nh+1)*512],
                                     start=(ic == 0), stop=(ic == IC - 1))
                nc.scalar.activation(eb[:, nh*512:(nh+1)*512], afp[:], ACT.Exp)
                sr = dps.tile([K, 512], F32, tag="sr", bufs=1)
                nc.tensor.matmul(sr[0:1, :], wB[0:K, BW_ONES19:BW_ONES19 + 1],
                                 eb[:, nh*512:(nh+1)*512], start=True, stop=True)
                rr = dsb.tile([1, 512], BF16, tag="rr", bufs=2)
                nc.vector.reciprocal(rr[:], sr[0:1, :])
                sr2 = dps.tile([K, 512], F32, tag="sr", bufs=1)
                nc.tensor.matmul(sr2[:], wB[0:1, BW_ONER19:BW_ONER19 + K], rr[:],
                                 start=True, stop=True)
                nc.vector.tensor_mul(eb[:, nh*512:(nh+1)*512],
                                     eb[:, nh*512:(nh+1)*512], sr2[:])
            at = {}
            for ic in range(IC):
                for nh in range(2):
                    aop = dps.tile([128, 512], F32, tag="aop", bufs=2)
                    nc.tensor.matmul(aop[:], val[:, ic*128:(ic+1)*128],
                                     eb[:, nh*512:(nh+1)*512], start=True, stop=True)
                    a_ = dsb.tile([128, 512], BF16, tag="at", bufs=6)
                    col = D_RS + (ic * NB + n) * 2 + nh
                    nc.scalar.activation(a_[:], aop[:], ACT.Copy,
                                         accum_out=dsm[:, col:col + 1])
                    at[(ic, nh)] = a_
            for cc in range(CC):
                for nh in range(2):
                    yp = dps.tile([128, 512], F32, tag="yp", bufs=2)
                    for ic in range(IC):
                        nc.tensor.matmul(yp[:], outwT[:, ic*C + cc*128:ic*C + (cc+1)*128],
                                         at[(ic, nh)][:], start=(ic == 0), stop=(ic == IC - 1))
                    nc.vector.tensor_copy(ybin(cc, n, nh), yp[:])
                    ysq = dsb.tile([128, 512], BF16, tag="ysq", bufs=2)
                    col = D_SQ + (cc * NB + n) * 2 + nh
                    nc.scalar.activation(ysq[:], yp[:], ACT.Square,
                                         accum_out=dsm[:, col:col + 1])

    # ---------------- phase E: finish stats ----------------
    with tc.tile_pool(name="phE_ps", bufs=1, space="PSUM") as eps_:
        for ic in range(IC):
            nc.vector.tensor_reduce(dsm[:, D_RSUM + ic:D_RSUM + ic + 1],
                                    dsm[:, D_RS + ic*2*NB:D_RS + (ic+1)*2*NB],
                                    axis=AX.X, op=OP.add)
        nc.vector.tensor_copy(rsb[:, 0:IC], dsm[:, D_RSUM:D_RSUM + IC])
        for cc in range(CC):
            mp = eps_.tile([128, 1], F32, tag="mup", bufs=2)
            for ic in range(IC):
                nc.tensor.matmul(mp[:], outwT[:, ic*C + cc*128:ic*C + (cc+1)*128],
                                 rsb[:, ic:ic + 1], start=(ic == 0), stop=(ic == IC - 1))
            nc.vector.tensor_copy(dsm[:, D_ST + 2*cc:D_ST + 2*cc + 1], mp[:])
            nc.vector.tensor_reduce(dsm[:, D_ST + 2*cc + 1:D_ST + 2*cc + 2],
                                    dsm[:, D_SQ + cc*2*NB:D_SQ + (cc+1)*2*NB],
                                    axis=AX.X, op=OP.add)

    # ---------------- collective ----------------
    with tc.tile_pool(name="cdram", bufs=1, space="DRAM") as cdram:
        arin = cdram.tile([128, 2 * CC], F32)
        arout = cdram.tile([128, 2 * CC], F32)
        nc.sync.dma_start(arin[:], dsm[:, D_ST:D_ST + 2 * CC])
        if collective:
            nc.gpsimd.collective_compute(
                "AllReduce", OP.add,
                ins=[arin.opt()], outs=[arout.opt()],
                replica_groups=[list(range(n_cores))],
            )
            nc.sync.dma_start(dsm[:, D_SBN:D_SBN + 2 * CC], arout[:])
        else:
            nc.sync.dma_start(dsm[:, D_SBN:D_SBN + 2 * CC], arin[:])

    # ---------------- BN finalize ----------------
    mom = dsm[:, D_MOM:D_MOM + 2 * CC]
    nc.scalar.mul(mom, dsm[:, D_SBN:D_SBN + 2 * CC], 1.0 / Ntot)
    muv = mom.rearrange("p (c two) -> p c two", two=2)[:, :, 0]
    msq = mom.rearrange("p (c two) -> p c two", two=2)[:, :, 1]
    nc.vector.tensor_mul(dsm[:, D_MUSQ:D_MUSQ + CC], muv, muv)
    nc.vector.tensor_sub(dsm[:, D_VAR:D_VAR + CC], msq, dsm[:, D_MUSQ:D_MUSQ + CC])
    nc.scalar.activation(dsm[:, D_SD:D_SD + CC], dsm[:, D_VAR:D_VAR + CC], ACT.Sqrt,
                         bias=wF[:, F_EPS:F_EPS + 1])
    nc.vector.reciprocal(dsm[:, D_RSTD:D_RSTD + CC], dsm[:, D_SD:D_SD + CC])
    scol = dsm[:, D_SCOL:D_SCOL + CC]
    bcol = dsm[:, D_BCOL:D_BCOL + CC]
    nc.vector.tensor_mul(scol, wF[:, F_GAMMA:F_GAMMA + CC], dsm[:, D_RSTD:D_RSTD + CC])
    nc.vector.tensor_scalar_mul(dsm[:, D_NSC:D_NSC + CC], scol, -1.0)
    for cc in range(CC):
        nc.vector.scalar_tensor_tensor(bcol[:, cc:cc + 1], muv[:, cc:cc + 1],
                                       dsm[:, D_NSC + cc:D_NSC + cc + 1],
                                       wF[:, F_BETA + cc:F_BETA + cc + 1], OP.mult, OP.add)

    # ---------------- phase F: affine + prelu + residual ----------------
    yv = y_d
    NCH = 4
    CHW = HWp // NCH
    with tc.tile_pool(name="phF_sb", bufs=1) as fsb:
        for cc in range(CC):
            for ch in range(NCH):
                ysl = y_sb[cc][:, ch*CHW:(ch+1)*CHW]
                u = fsb.tile([128, CHW], BF16, tag="u_f", bufs=2)
                nc.scalar.activation(u[:], ysl, ACT.Identity,
                                     bias=bcol[:, cc:cc + 1], scale=scol[:, cc:cc + 1])
                m = fsb.tile([128, CHW], BF16, tag="m_f", bufs=2)
                nc.gpsimd.tensor_scalar_min(m[:], u[:], 0.0)
                v = fsb.tile([128, CHW], BF16, tag="v_f", bufs=2)
                nc.vector.scalar_tensor_tensor(v[:], m[:], wB[:, BW_OAM1 + cc:BW_OAM1 + cc + 1],
                                               u[:], OP.mult, OP.add)
                o = fsb.tile([128, CHW], BF16, tag="o_f", bufs=3)
                nc.vector.tensor_add(o[:], v[:], x_sb[cc][:, ch*CHW:(ch+1)*CHW])
                nc.sync.dma_start(yv[cc*128:(cc+1)*128, ch*CHW:(ch+1)*CHW], o[:])

    spool.release()
    ypool.release()
    xpool.release()
    wpool.release()


# ======================================================================
# Entry point: kernel(**inputs) -> np.ndarray [8, 512, 64, 128]
# ======================================================================
import concourse.bacc as bacc
import concourse.tile as tile
from concourse.bass_utils import run_bass_kernel_spmd

N_CORES = 8
_cached = {}


def permute_x(xc):
    """[C, H, W] -> bin-blocked [C, HWp]: p = (bi*BW+bj)*P + hh*RW + ww."""
    return np.ascontiguousarray(
        xc.reshape(C, BH, RH, BW, RW).transpose(0, 1, 3, 2, 4).reshape(C, HWp))


def unpermute_y(yc):
    """bin-blocked [C, HWp] -> [C, H, W]."""
    return yc.reshape(C, BH, BW, RH, RW).transpose(0, 1, 3, 2, 4).reshape(C, H, W)


def _build_program(n_cores=N_CORES):
    if "nc" in _cached:
        return _cached["nc"]
    nc = bacc.Bacc("TRN2", target_bir_lowering=False, debug=False, num_devices=n_cores)
    ins = {"x": nc.dram_tensor("x", [C, HWp], BF16, kind="ExternalInput").ap()}
    for nm, shape, dt in WEIGHT_SPECS:
        ins[nm] = nc.dram_tensor(nm, shape, dt, kind="ExternalInput").ap()
    outs = {"y": nc.dram_tensor("y", [C, HWp], BF16, kind="ExternalOutput").ap()}
    with tile.TileContext(nc) as tc:
        build_caam(tc, outs, ins, n_cores)
    nc.compile()
    _cached["nc"] = nc
    return nc


def make_in_maps(inputs):
    x = np.asarray(inputs["x"], np.float32)
    prep = host_prep(inputs)
    in_maps = []
    for c in range(N_CORES):
        d = {"x": permute_x(x[c]).astype(NPBF)}
        for nm, _, _ in WEIGHT_SPECS:
            d[nm] = prep[nm]
        in_maps.append(d)
    return in_maps


def kernel(**inputs):
    nc = _build_program()
    in_maps = make_in_maps(inputs)
    res = run_bass_kernel_spmd(nc, in_maps, core_ids=list(range(N_CORES)))
    return np.stack([
        unpermute_y(np.asarray(res.results[c]["y"]).astype(np.float32))
        for c in range(N_CORES)
    ])
